# revision 18
# baseline (speedup 1.0000x reference)
"""Trainium2 Bass kernel for nn_CLIP_Embedding_35613868818658.

CNN stem (3x conv1d+GroupNorm+ReLU, 768->128->256->512) -> LayerNorm ->
bidirectional Mamba (selective scan, d_inner=1024, d_state=16, L=1024) ->
out_proj + residual.  Output (2, 512, 1024) f32.

Sharding: 2 batch-groups x 4-way d_inner split (DSH=256 rows per core).
Cores 0-3 handle b=0, cores 4-7 handle b=1; core g within a group owns
d_inner rows [256g, 256(g+1)).  Two in-group AllReduces: x_dbl (B/C/dt
projections, contracted over sharded d_inner) and the final out_proj.

Engine plan for the scan: softplus(dt) -> SP (scalar); a_s = exp(-(s+1)SP)
(scalar); b_s = (SP*u) .* B_s (vector tt, bf16 2x mode); h_s = hardware
tensor_tensor_scan (vector); g_s = h_s .* C_s (gpsimd); sum_s via identity
matmuls accumulated in PSUM (tensor engine).  B_s/C_s rows are broadcast
to 128 partitions by DMA from DRAM.  fwd and rev run as separate tiles so
each direction's AllReduce overlaps the other direction's compute.
"""

import numpy as np
import ml_dtypes

import concourse.bass as bass
import concourse.mybir as mybir
import concourse.tile as tile
from contextlib import ExitStack

BF16 = ml_dtypes.bfloat16
F32 = mybir.dt.float32
BF = mybir.dt.bfloat16

B, CIN, L = 2, 768, 1024
DM, DI, DS, DTR, DC = 512, 1024, 16, 32, 4
NCORES, NGRP = 8, 4
DSH = DI // NGRP          # 256 d_inner rows per core
NDT = DSH // 128          # 2 d-tiles of 128 partitions
EPS = 1e-5

AluOp = mybir.AluOpType
ActFn = mybir.ActivationFunctionType


def _ap_bcast_dram(handle, offset, dims):
    """Raw AP on a DRAM tensor: dims is a list of [step, count]."""
    return bass.AP(tensor=handle, offset=offset, ap=[list(d) for d in dims])


def split_excess_waits(nc, max_waits=1):
    """Walrus rejects instructions carrying more sync waits than the ISA
    encoding has slots for (1 on this toolchain).  Move excess waits onto
    preceding same-engine NoOps."""
    for bb in nc.main_func.blocks:
        insts = bb.instructions
        out, changed = [], False
        for ins in insts:
            si = ins.sync_info
            if si is not None and si.on_wait is not None and len(si.on_wait) > max_waits:
                waits = list(si.on_wait)
                keep, rest = waits[:max_waits], waits[max_waits:]
                idx = 0
                while rest:
                    chunk, rest = rest[:max_waits], rest[max_waits:]
                    nop = mybir.InstNoOp(
                        name=f"{ins.name}-wsplit{idx}",
                        engine=ins.engine,
                        sync_info=mybir.SyncInfo(on_wait=chunk, on_update=[]),
                        bass_nofuse=True,
                    )
                    out.append(nop)
                    idx += 1
                ins.sync_info = mybir.SyncInfo(
                    on_wait=keep, on_update=list(si.on_update or [])
                )
                changed = True
            out.append(ins)
        if changed:
            bb.instructions = out


def build_program(a_vals, split_waits=True, reps=1, upto='full'):
    """Build the SPMD Bass program.  a_vals: 16 negative floats, A[s] = -(s+1)
    (verified d-independent and equal for both directions on the host)."""
    nc = bass.Bass("TRN2", target_bir_lowering=False, debug=False,
                   num_devices=NCORES)

    dt_in = lambda n, s, d=BF: nc.dram_tensor(n, list(s), d, kind="ExternalInput")

    x_in = dt_in("x", (CIN, L + 2))                      # host-padded, bf16
    w1T = dt_in("w1T", (3, 6, 128, 128))
    w2T = dt_in("w2T", (3, 1, 128, 256))
    w3T = dt_in("w3T", (3, 2, 128, 512))
    colpack = dt_in("colpack", (128, 33), F32)
    onehot = dt_in("onehot", (3, 128, 32))
    ones_col = dt_in("ones_col", (128, 1))
    inprojT = dt_in("inprojT", (4, 128, 512))
    augT = dt_in("augT", (2, 512))
    xpT = dt_in("xpT", (2, 2, 128, 64))                 # [dir][ktile]
    dtT = dt_in("dtT", (2, 32, 256))                    # [dir]
    dwdiag = dt_in("dwdiag", (2, 2, 4, 128, 128))       # [dir][dt][k] diag
    outT = dt_in("outT", (2, 128, 512))                 # [dtile]
    id128 = dt_in("id128", (128, 128))

    out_ext = nc.dram_tensor("out", [DM, L], BF, kind="ExternalOutput")

    with tile.TileContext(nc) as tc, ExitStack() as ctx:
        P = 128
        consts = ctx.enter_context(tc.tile_pool(name="consts", bufs=1))
        psum = ctx.enter_context(tc.tile_pool(name="psum", bufs=2, space="PSUM"))
        ypsum = ctx.enter_context(tc.tile_pool(name="ypsum", bufs=1, space="PSUM"))
        mid = ctx.enter_context(tc.tile_pool(name="mid", bufs=1))
        dram = ctx.enter_context(tc.tile_pool(name="dram", bufs=1, space="DRAM"))
        sync, vec, pool, act, pe = nc.sync, nc.vector, nc.gpsimd, nc.scalar, nc.tensor

        # ---------------- consts to SBUF ----------------
        _ldq = [0]
        def load(poolh, shape, src, dtype=BF, name=None):
            t = poolh.tile(list(shape), dtype, tag=name)
            eng = sync if _ldq[0] % 2 == 0 else act
            _ldq[0] += 1
            eng.dma_start(t[:], src)
            return t

        def load_packed(dramt, cols, blocks, blkstep, name, dtype=BF):
            """One DMA: dram tensor of `blocks` [128, cols//blocks] tiles ->
            SBUF [128, cols].  blkstep = elements per block (128*width)."""
            t = consts.tile([P, cols], dtype, tag=name)
            width = cols // blocks
            base = dramt[tuple([0] * (len(dramt.shape) - 2))]
            src_ap = bass.AP(tensor=base.tensor, offset=0,
                             ap=[[width, 128], [blkstep, blocks], [1, width]])
            eng = sync if _ldq[0] % 2 == 0 else act
            _ldq[0] += 1
            eng.dma_start(t[:], src_ap)
            return t

        w1b = load_packed(w1T, 18 * 128, 18, 128 * 128, "w1b")
        w1 = [[w1b[:, (k * 6 + ct) * 128:(k * 6 + ct + 1) * 128]
               for ct in range(6)] for k in range(3)]
        w2b = load_packed(w2T, 3 * 256, 3, 128 * 256, "w2b")
        w2 = [[w2b[:, k * 256:(k + 1) * 256]] for k in range(3)]
        w3b = load_packed(w3T, 6 * 512, 6, 128 * 512, "w3b")
        w3 = [[w3b[:, (k * 2 + ct) * 512:(k * 2 + ct + 1) * 512]
               for ct in range(2)] for k in range(3)]
        onehb = load_packed(onehot, 3 * 32, 3, 128 * 32, "onehb")
        oneh = [onehb[:, i * 32:(i + 1) * 32] for i in range(3)]
        ipb = load_packed(inprojT, 4 * 512, 4, 128 * 512, "ipb")
        ipT = [ipb[:, kt * 512:(kt + 1) * 512] for kt in range(4)]
        xpb = load_packed(xpT, 4 * 64, 4, 128 * 64, "xpb")
        xpTs = [[xpb[:, (d * 2 + kt) * 64:(d * 2 + kt + 1) * 64]
                 for kt in range(2)] for d in range(2)]
        dwb = load_packed(dwdiag, 16 * 128, 16, 128 * 128, "dwb")
        dwds = [[[dwb[:, ((d * 2 + dt) * 4 + k) * 128:((d * 2 + dt) * 4 + k + 1) * 128]
                  for k in range(4)] for dt in range(2)] for d in range(2)]
        outb = load_packed(outT, 2 * 512, 2, 128 * 512, "outb")
        outTs = [outb[:, dt * 512:(dt + 1) * 512] for dt in range(2)]
        colsb = load(consts, (P, 33), colpack[:], F32, "colsb")
        _co = [0]
        def cols_take(n):
            lo = _co[0]; _co[0] += n
            return [colsb[:, lo + i:lo + i + 1] for i in range(n)]
        cbs = [cols_take(1), cols_take(2), cols_take(4)]
        gngs = [cols_take(1), cols_take(2), cols_take(4)]
        gnbs = [cols_take(1), cols_take(2), cols_take(4)]
        _dtb_c = cols_take(4)
        dtbs = [[_dtb_c[d * 2 + dt] for dt in range(2)] for d in range(2)]
        _cvb_c = cols_take(4)
        cvbs = [[_cvb_c[d * 2 + dt] for dt in range(2)] for d in range(2)]
        _D_c = cols_take(4)
        Dcols = [[_D_c[d * 2 + dt] for dt in range(2)] for d in range(2)]
        ones1 = load(consts, (P, 1), ones_col[:], name="ones1")
        augTs = load(consts, (2, 512), augT[:], name="augT")
        dtTs = [load(consts, (32, 256), dtT[d], name=f"dtT{d}") for d in range(2)]
        id128s = load(consts, (P, 128), id128[:], name="id128")

        epsc = consts.tile([128, 1], F32, tag="epsc")
        vec.memset(epsc[:], EPS)

        # DRAM scratch
        gn_scrs = {1: dram.tile([32, 2], F32, tag="gn_scr1", name="gn_scr1"),
                   2: dram.tile([32, 4], F32, tag="gn_scr2", name="gn_scr2"),
                   4: dram.tile([32, 8], F32, tag="gn_scr4", name="gn_scr4")}
        ln_scr = dram.tile([1, L], F32, tag="ln_scr")
        xdbl_loc = dram.tile([2, 64, L], BF, tag="xdbl_loc")
        xdbl_red = dram.tile([2, 64, L], BF, tag="xdbl_red")
        out_loc = dram.tile([DM, L], BF, tag="out_loc")
        out_red = dram.tile([DM, L], BF, tag="out_red")

        for rep in range(reps):
            fctx = ExitStack()
            stem = fctx.enter_context(tc.tile_pool(name=f"stem{rep}", bufs=1))
            stemtmp = fctx.enter_context(tc.tile_pool(name=f"stemtmp{rep}", bufs=3))
            statp = fctx.enter_context(tc.tile_pool(name=f"statp{rep}", bufs=2))
            rows = fctx.enter_context(tc.tile_pool(name=f"rows{rep}", bufs=1))
            xall = stem.tile([P, 6 * (L + 2)], BF, tag="xall")
            sync.dma_start(
                xall[:],
                bass.AP(tensor=x_in[:].tensor, offset=0,
                        ap=[[L + 2, 128], [128 * (L + 2), 6], [1, L + 2]]),
            )
            x_t = [xall[:, ct * (L + 2):(ct + 1) * (L + 2)] for ct in range(6)]

            # ---------------- CNN stem ----------------
            def conv_gn_relu(layer, in_tiles, ws, cb, gng, gnb, co, out_f32):
                """in_tiles: list of padded (128, L+2) bf16.  Batched GN stat
                path: one Rsqrt + one DRAM round-trip per layer."""
                n_ct = len(in_tiles)
                n_co = co // 128
                cg = co // 32            # channels per group
                ngt = 128 // cg          # groups per 128-channel tile
                group_elems = float(cg) * L
                h_raws, outs = [], []
                vars_t = statp.tile([32, n_co], F32, tag="vars_t")
                means_t = statp.tile([32, n_co], F32, tag="means_t")
                for mt in range(n_co):
                    h_raw = stem.tile([P, L], F32, tag=f"h_raw{mt}")
                    stat4 = statp.tile([P, 4], F32, tag="stat4")
                    sq = stemtmp.tile([P, 512], BF, tag="sq")
                    for n in range(2):
                        ps = psum.tile([P, 512], F32, tag="ps_main", name="ps")
                        nmm = n_ct * 3
                        i = 0
                        for ct in range(n_ct):
                            for k in range(3):
                                pe.matmul(
                                    ps[:],
                                    ws[k][ct][:, mt * 128:(mt + 1) * 128],
                                    in_tiles[ct][:, n * 512 + k: n * 512 + k + 512],
                                    start=(i == 0), stop=(i == nmm - 1),
                                )
                                i += 1
                        act.activation(h_raw[:, n * 512:(n + 1) * 512], ps[:],
                                       ActFn.Identity, bias=cb[mt],
                                       accum_out=stat4[:, n:n + 1])
                        act.activation(sq[:], h_raw[:, n * 512:(n + 1) * 512],
                                       ActFn.Square, accum_out=stat4[:, 2 + n:3 + n])
                    # group stats: per-partition sums -> per-group via one-hot matmul
                    stat4b = statp.tile([P, 4], BF, tag="stat4b")
                    vec.tensor_copy(stat4b[:], stat4[:])
                    gps = psum.tile([32, 4], F32, tag="ps_small", name="gps", bufs=2)
                    pe.matmul(gps[:], oneh[layer - 1], stat4b[:])
                    gsb = statp.tile([32, 4], F32, tag="gsb")
                    vec.tensor_copy(gsb[:], gps[:])
                    sx = statp.tile([32, 1], F32, tag="sx")
                    sq_g = statp.tile([32, 1], F32, tag="sq_g")
                    vec.tensor_add(sx[:], gsb[:, 0:1], gsb[:, 1:2])
                    vec.tensor_add(sq_g[:], gsb[:, 2:3], gsb[:, 3:4])
                    vec.tensor_scalar_mul(means_t[:, mt:mt + 1], sx[:],
                                          1.0 / group_elems)
                    msq = statp.tile([32, 1], F32, tag="msq")
                    vec.tensor_mul(msq[:], means_t[:, mt:mt + 1],
                                   means_t[:, mt:mt + 1])
                    vec.scalar_tensor_tensor(vars_t[:, mt:mt + 1], sq_g[:],
                                             1.0 / group_elems, msq[:],
                                             AluOp.mult, AluOp.subtract)
                    h_raws.append(h_raw)
                # one Sqrt for the whole layer, one DRAM round trip
                sig_l = statp.tile([32, n_co], F32, tag="sig_l")
                act.activation(sig_l[:], vars_t[:], ActFn.Sqrt, bias=epsc[:32, :])
                rstds = statp.tile([32, n_co], F32, tag="rstds")
                vec.reciprocal(rstds[:], sig_l[:])
                stat2 = statp.tile([32, 2 * n_co], F32, tag="stat2")
                for mt in range(n_co):
                    vec.tensor_copy(stat2[:, 2 * mt:2 * mt + 1], rstds[:, mt:mt + 1])
                    vec.tensor_copy(stat2[:, 2 * mt + 1:2 * mt + 2],
                                    means_t[:, mt:mt + 1])
                gn_scr = gn_scrs[n_co]
                sync.dma_start(gn_scr[:], stat2[:])
                for mt in range(n_co):
                    ch2 = statp.tile([P, 2], F32, tag="ch2")
                    sync.dma_start(
                        ch2[:],
                        _ap_bcast_dram(gn_scr[:].tensor,
                                       gn_scr[:].offset + 2 * mt,
                                       [[2 * n_co, ngt], [0, cg], [1, 2]]),
                    )
                    scale_c = statp.tile([P, 1], F32, tag="scale_c")
                    vec.tensor_mul(scale_c[:], ch2[:, 0:1], gng[mt])
                    nmean_s = statp.tile([P, 1], F32, tag="nmean_s")
                    vec.tensor_mul(nmean_s[:], ch2[:, 1:2], scale_c[:])
                    bias_c = statp.tile([P, 1], F32, tag="bias_c")
                    vec.tensor_sub(bias_c[:], gnb[mt], nmean_s[:])
                    h_raw = h_raws[mt]
                    if out_f32:
                        h_out = mid.tile([P, L], F32, tag=f"res{mt}")
                        act.activation(h_out[:], h_raw[:], ActFn.Relu,
                                       scale=scale_c[:], bias=bias_c[:])
                    else:
                        h_out = stem.tile([P, L + 2], BF, tag=f"h{layer}_{mt}")
                        vec.memset(h_out[:, 0:1], 0.0)
                        vec.memset(h_out[:, L + 1:L + 2], 0.0)
                        act.activation(h_out[:, 1:L + 1], h_raw[:], ActFn.Relu,
                                       scale=scale_c[:], bias=bias_c[:])
                    outs.append(h_out)
                return outs

            h1 = conv_gn_relu(1, x_t, w1, cbs[0], gngs[0], gnbs[0], 128, False)
            h2 = conv_gn_relu(2, h1, w2, cbs[1], gngs[1], gnbs[1], 256, False)
            res = conv_gn_relu(3, h2, w3, cbs[2], gngs[2], gnbs[2], 512, True)

            h3b = []
            for mt in range(4):
                t = stem.tile([P, L], BF, tag=f"h3b{mt}")
                vec.tensor_copy(t[:], res[mt][:])
                h3b.append(t)

            if upto == 'stem':
                sync.dma_start(out_ext[0:128, :], res[0][:])
                fctx.close()
                continue
            # ---------------- LayerNorm stats (over channels, via matmuls) -------
            hsq = []
            for mt in range(4):
                t = stemtmp.tile([P, L], BF, tag="hsq")
                act.activation(t[:], h3b[mt][:], ActFn.Square)
                hsq.append(t)
            musum = rows.tile([1, L], F32, tag="musum")
            sqsum = rows.tile([1, L], F32, tag="sqsum")
            for n in range(2):
                mu_ps = psum.tile([1, 512], F32, tag="ps_main", name="mu_ps")
                for kt in range(4):
                    pe.matmul(mu_ps[:], ones1[:],
                              h3b[kt][:, n * 512:(n + 1) * 512],
                              start=(kt == 0), stop=(kt == 3))
                vec.tensor_copy(musum[:, n * 512:(n + 1) * 512], mu_ps[:])
                sq_ps = psum.tile([1, 512], F32, tag="ps_main", name="sq_ps")
                for kt in range(4):
                    pe.matmul(sq_ps[:], ones1[:],
                              hsq[kt][:, n * 512:(n + 1) * 512],
                              start=(kt == 0), stop=(kt == 3))
                vec.tensor_copy(sqsum[:, n * 512:(n + 1) * 512], sq_ps[:])
            nmu = rows.tile([1, L], F32, tag="nmu")
            vec.tensor_scalar_mul(nmu[:], musum[:], -1.0 / DM)
            msql = rows.tile([1, L], F32, tag="msql")
            vec.tensor_mul(msql[:], nmu[:], nmu[:])
            varl = rows.tile([1, L], F32, tag="varl")
            vec.scalar_tensor_tensor(varl[:], sqsum[:], 1.0 / DM, msql[:],
                                     AluOp.mult, AluOp.subtract)
            sigma = rows.tile([1, L], F32, tag="sigma")
            act.activation(sigma[:], varl[:], ActFn.Sqrt, bias=epsc[:1, :])
            recip = rows.tile([1, L], F32, tag="recip")
            vec.reciprocal(recip[:], sigma[:])
            nmu_b = rows.tile([1, L], BF, tag="nmu_b")
            vec.tensor_copy(nmu_b[:], nmu[:])
            sig_b = rows.tile([1, L], BF, tag="sig_b")
            vec.tensor_copy(sig_b[:], sigma[:])
            aug = rows.tile([2, L], BF, tag="aug")
            sync.dma_start(aug[0:1, :], nmu_b[:])
            sync.dma_start(aug[1:2, :], sig_b[:])
            sync.dma_start(ln_scr[:], recip[:])
            rbc = rows.tile([P, L], F32, tag="rbc")
            sync.dma_start(
                rbc[:],
                _ap_bcast_dram(ln_scr[:].tensor, ln_scr[:].offset, [[0, P], [1, L]]),
            )

            # ---------------- in_proj (LN folded in) ----------------
            # xpad[dt]: (128, L+6) bf16, 3 zero cols each side; z[dt]: (128, L)
            xpad = []
            zt = []
            for dt in range(NDT):
                xp_ = mid.tile([P, L + 6], BF, tag=f"xpad{dt}")
                vec.memset(xp_[:, 0:3], 0.0)
                vec.memset(xp_[:, L + 3:L + 6], 0.0)
                xpad.append(xp_)
                zt.append(mid.tile([P, L], BF, tag=f"z{dt}", name=f"z{dt}"))
            for m in range(4):
                for n in range(2):
                    ps = psum.tile([P, 512], F32, tag="ps_main", name="ps")
                    for kt in range(4):
                        pe.matmul(ps[:], ipT[kt][:, m * 128:(m + 1) * 128],
                                  h3b[kt][:, n * 512:(n + 1) * 512],
                                  start=(kt == 0), stop=False)
                    pe.matmul(ps[:], augTs[:, m * 128:(m + 1) * 128],
                              aug[:, n * 512:(n + 1) * 512], start=False, stop=True)
                    if m < 2:
                        dst = xpad[m][:, 3 + n * 512: 3 + (n + 1) * 512]
                    else:
                        dst = zt[m - 2][:, n * 512:(n + 1) * 512]
                    vec.tensor_mul(dst, ps[:], rbc[:, n * 512:(n + 1) * 512])

            if upto == 'inproj':
                sync.dma_start(out_ext[0:128, :], res[0][:])
                fctx.close()
                continue
            fctx.close()  # free stem/LN scratch address space for the scan phase
            sctx = ExitStack()
            scanp = sctx.enter_context(tc.tile_pool(name=f"scanp{rep}", bufs=6))
            onep = sctx.enter_context(tc.tile_pool(name=f"onep{rep}", bufs=1))
            twop = sctx.enter_context(tc.tile_pool(name=f"twop{rep}", bufs=2))

            # u[dir][dt]: depthwise causal conv + silu on the tensor engine.
            # rev domain is time-reversed (tau = L-1-t).
            u = [[None, None], [None, None]]

            def dwconv(d):
                for dt in range(NDT):
                    if d == 0:
                        udst = mid.tile([P, L], BF, tag=f"u0{dt}", name=f"u0{dt}")
                    else:
                        utmp = twop.tile([P, L], BF, tag="utmp")
                    for c in range(2):
                        ps = psum.tile([P, 512], F32, tag="ps_main", name="ps")
                        for k in range(4):
                            off = c * 512 + k + (3 if d == 1 else 0)
                            pe.matmul(ps[:], dwds[d][dt][k],
                                      xpad[dt][:, off:off + 512],
                                      start=(k == 0), stop=(k == 3))
                        dst = udst if d == 0 else utmp
                        xb = twop.tile([P, 512], BF, tag="dwxb")
                        act.activation(xb[:], ps[:], ActFn.Identity,
                                       bias=cvbs[d][dt])
                        sg = twop.tile([P, 512], BF, tag="dwsg")
                        act.activation(sg[:], ps[:], ActFn.Sigmoid,
                                       bias=cvbs[d][dt])
                        vec.tensor_mul(dst[:, c * 512:(c + 1) * 512], xb[:], sg[:])
                    if d == 1:
                        udst = mid.tile([P, L], BF, tag=f"u1{dt}", name=f"u1{dt}")
                        vec.tensor_copy(udst[:], utmp[:, L - 1::-1])
                    u[d][dt] = udst

            def xdbl_proj(d):
                xsb = twop.tile([64, L], BF, tag="xsb")
                for n in range(2):
                    xps = psum.tile([64, 512], F32, tag="ps_main", name="xps")
                    for dt in range(NDT):
                        pe.matmul(xps[:], xpTs[d][dt],
                                  u[d][dt][:, n * 512:(n + 1) * 512],
                                  start=(dt == 0), stop=(dt == 1))
                    vec.tensor_copy(xsb[:, n * 512:(n + 1) * 512], xps[:])
                sync.dma_start(xdbl_loc[d], xsb[:])
                pool.collective_compute(
                    "AllReduce", AluOp.add,
                    replica_groups=[[0, 1, 2, 3], [4, 5, 6, 7]],
                    ins=[xdbl_loc[d].opt()],
                    outs=[xdbl_red[d].opt()],
                )

            dwconv(0)
            xdbl_proj(0)          # AR(fwd) in flight...
            dwconv(1)             # ...while rev dwconv runs
            xdbl_proj(1)
            # silu(z) gating precompute (silu table still loaded)
            zs = []
            for dt in range(NDT):
                sgz = twop.tile([P, L], BF, tag="sgz")
                act.activation(sgz[:], zt[dt][:], ActFn.Sigmoid)
                zs_t = onep.tile([P, L], BF, tag=f"zs{dt}", name=f"zs{dt}")
                vec.tensor_mul(zs_t[:], zt[dt][:], sgz[:])
                zs.append(zs_t)

            if upto == 'dw':
                sync.dma_start(out_ext[0:128, :], res[0][:])
                sctx.close()
                continue

            # ---------------- per-direction: dt_proj, B/C rows, scan ---------
            SP = [[None, None], [None, None]]
            mx = [[None, None], [None, None]]
            y_sb = [[None, None], [None, None]]

            def dt_bc(d):
                """softplus(dt_proj) -> SP, mx = SP*u; B/C rows -> bmc DRAM."""
                dtfb = twop.tile([32, L], BF, tag="dtfb", name=f"dtfb{d}", bufs=1)
                sync.dma_start(dtfb[:], xdbl_red[d, 0:32, :])
                for dt in range(NDT):
                    sp_t = mid.tile([P, L], BF, tag=f"sp{d}{dt}", name=f"sp{d}{dt}")
                    for n in range(2):
                        ps = psum.tile([P, 512], F32, tag="ps_main", name="ps")
                        pe.matmul(ps[:], dtTs[d][:, dt * 128:(dt + 1) * 128],
                                  dtfb[:, n * 512:(n + 1) * 512])
                        sgm = twop.tile([P, 512], F32, tag="sgm")
                        act.activation(sgm[:], ps[:], ActFn.Sigmoid, scale=-1.0,
                                       bias=dtbs[d][dt])
                        act.activation(sp_t[:, n * 512:(n + 1) * 512], sgm[:],
                                       ActFn.Ln)
                    SP[d][dt] = sp_t
                    mx_t = mid.tile([P, L], BF, tag=f"mx{d}{dt}", name=f"mx{d}{dt}")
                    vec.scalar_tensor_tensor(mx_t[:], sp_t[:], -1.0, u[d][dt][:],
                                             AluOp.mult, AluOp.mult)
                    mx[d][dt] = mx_t

            def scan_dir(d):
                bmc_ap = xdbl_red[:]
                ys = [[ypsum.tile([P, 512], F32, tag=f"ys{dt}{c}", name=f"ys{dt}{c}")
                       for c in range(2)] for dt in range(NDT)]
                for s in range(16):
                    Bs = scanp.tile([P, L], BF, tag="Bs")
                    sync.dma_start(
                        Bs[:],
                        _ap_bcast_dram(bmc_ap.tensor,
                                       bmc_ap.offset + d * 64 * L + (32 + s) * L,
                                       [[0, P], [1, L]]),
                    )
                    Cs = scanp.tile([P, L], BF, tag="Cs")
                    sync.dma_start(
                        Cs[:],
                        _ap_bcast_dram(bmc_ap.tensor,
                                       bmc_ap.offset + d * 64 * L + (48 + s) * L,
                                       [[0, P], [1, L]]),
                    )
                    for dt in range(NDT):
                        a_s = scanp.tile([P, L], BF, tag="a_s")
                        act.activation(a_s[:], SP[d][dt][:], ActFn.Exp,
                                       scale=float(-a_vals[s]))
                        b_s = scanp.tile([P, L], BF, tag="b_s")
                        if s % 4 == 3:
                            pool.tensor_mul(b_s[:], mx[d][dt][:], Bs[:])
                        else:
                            vec.tensor_mul(b_s[:], mx[d][dt][:], Bs[:])
                        h_s = scanp.tile([P, L], BF, tag="h_s")
                        vec.tensor_tensor_scan(h_s[:], a_s[:], b_s[:], 0.0,
                                               AluOp.mult, AluOp.add)
                        gs = scanp.tile([P, L], BF, tag="gs")
                        pool.tensor_mul(gs[:], h_s[:], Cs[:])
                        for c in range(2):
                            pe.matmul(ys[dt][c][:], id128s[:],
                                      gs[:, c * 512:(c + 1) * 512],
                                      start=(s == 0), stop=(s == 15))
                for dt in range(NDT):
                    yb = twop.tile([P, L], BF, tag=f"yb{d}{dt}", name=f"yb{d}{dt}", bufs=1)
                    for c in range(2):
                        act.activation(yb[:, c * 512:(c + 1) * 512],
                                       ys[dt][c][:], ActFn.Copy)
                    y_sb[d][dt] = yb

            dt_bc(0)
            dt_bc(1)              # AR(rev) done by now; share act tables
            scan_dir(0)
            scan_dir(1)

            if upto == 'scan':
                sync.dma_start(out_ext[0:128, :], res[0][:])
                sctx.close()
                continue
            # ---------------- combine directions, D-term, gate ----------------
            ygate = []
            for dt in range(NDT):
                ysum = twop.tile([P, L], BF, tag="ysum")
                vec.tensor_add(ysum[:], y_sb[0][dt][:], y_sb[1][dt][:, L - 1::-1])
                t1 = twop.tile([P, L], BF, tag="t1")
                vec.scalar_tensor_tensor(t1[:], u[0][dt][:],
                                         Dcols[0][dt], ysum[:],
                                         AluOp.mult, AluOp.add)
                t2 = twop.tile([P, L], BF, tag="t2")
                vec.scalar_tensor_tensor(t2[:], u[1][dt][:, L - 1::-1],
                                         Dcols[1][dt], t1[:],
                                         AluOp.mult, AluOp.add)
                yg = twop.tile([P, L], BF, tag="yg", name=f"yg{dt}")
                vec.tensor_mul(yg[:], t2[:], zs[dt][:])
                ygate.append(yg)

            # ---------------- out_proj + residual + chunked AllReduce ---------
            for m in range(4):
                osb = twop.tile([P, L], BF, tag="osb")
                for n in range(2):
                    ps = psum.tile([P, 512], F32, tag="ps_main", name="ps")
                    for dt in range(NDT):
                        pe.matmul(ps[:], outTs[dt][:, m * 128:(m + 1) * 128],
                                  ygate[dt][:, n * 512:(n + 1) * 512],
                                  start=(dt == 0), stop=(dt == 1))
                    vec.scalar_tensor_tensor(osb[:, n * 512:(n + 1) * 512],
                                             res[m][:, n * 512:(n + 1) * 512],
                                             1.0 / NGRP, ps[:],
                                             AluOp.mult, AluOp.add)
                sync.dma_start(out_loc[m * 128:(m + 1) * 128, :], osb[:])
                if m % 2 == 1:
                    lo, hi = (m - 1) * 128, (m + 1) * 128
                    pool.collective_compute(
                        "AllReduce", AluOp.add,
                        replica_groups=[[0, 1, 2, 3], [4, 5, 6, 7]],
                        ins=[out_loc[lo:hi, :].opt()],
                        outs=[out_red[lo:hi, :].opt()],
                    )
                    sync.dma_start(out_ext[lo:hi, :], out_red[lo:hi, :])
            sctx.close()

    if split_waits:
        split_excess_waits(nc)
    return nc


def prep_inputs(inputs):
    """Host-side sharding/weight prep.  Returns (a_vals, in_maps)."""
    f32 = lambda a: np.ascontiguousarray(np.asarray(a, np.float32))
    bf = lambda a: np.ascontiguousarray(np.asarray(a, np.float32).astype(BF16))

    A_f = -np.exp(f32(inputs["Alog_f"]))
    A_r = -np.exp(f32(inputs["Alog_r"]))
    assert np.abs(A_f - A_f[0:1]).max() < 1e-5, "A not d-independent"
    assert np.abs(A_f - A_r).max() < 1e-5, "A_f != A_r"
    a_vals = [float(v) for v in A_f[0]]

    x = f32(inputs["x"])
    w1 = f32(inputs["conv1_w"]); w2 = f32(inputs["conv2_w"]); w3 = f32(inputs["conv3_w"])
    w1T = bf(np.transpose(w1, (2, 1, 0)).reshape(3, 6, 128, 128))
    w2T = bf(np.transpose(w2, (2, 1, 0)).reshape(3, 1, 128, 256))
    w3T = bf(np.transpose(w3, (2, 1, 0)).reshape(3, 2, 128, 512))
    onehot = np.zeros((3, 128, 32), np.float32)
    for i, cg in enumerate((4, 8, 16)):
        onehot[i, np.arange(128), np.arange(128) // cg] = 1.0
    ln_g = f32(inputs["ln_g"]); ln_b = f32(inputs["ln_b"])
    ipw = f32(inputs["in_proj_w"])
    opw = f32(inputs["out_proj_w"])

    def colsplit(a):
        a = f32(a).reshape(-1)
        return [a[i * 128:(i + 1) * 128].reshape(128, 1) for i in range(len(a) // 128)]

    common_cols = (colsplit(inputs["conv1_b"]) + colsplit(inputs["conv2_b"]) +
                   colsplit(inputs["conv3_b"]) + colsplit(inputs["gn1_g"]) +
                   colsplit(inputs["gn2_g"]) + colsplit(inputs["gn3_g"]) +
                   colsplit(inputs["gn1_b"]) + colsplit(inputs["gn2_b"]) +
                   colsplit(inputs["gn3_b"]))
    common = dict(
        w1T=w1T, w2T=w2T, w3T=w3T,
        onehot=bf(onehot),
        ones_col=bf(np.ones((128, 1), np.float32)),
        id128=bf(np.eye(128, dtype=np.float32)),
    )

    in_maps = []
    for core in range(NCORES):
        b, grp = core // NGRP, core % NGRP
        rows = np.arange(grp * DSH, (grp + 1) * DSH)
        sel = np.concatenate([rows, DI + rows])
        Wsel = ipw[sel] * ln_g[None, :]
        inprojT = bf(Wsel.T.reshape(4, 128, 2 * DSH))
        augTm = bf(np.stack([Wsel.sum(1), ipw[sel] @ ln_b]))
        xpTm = np.stack([
            bf(f32(inputs[f"xp_w_{s}"])[:, rows].T.reshape(2, 128, 64))
            for s in ("f", "r")])
        dtTm = np.stack([
            bf(f32(inputs[f"dt_w_{s}"])[rows].T) for s in ("f", "r")])
        dtb_cols = [(-f32(inputs[f"dt_b_{s}"])[rows])[dt * 128:(dt + 1) * 128]
                    .reshape(128, 1) for s in ("f", "r") for dt in range(2)]
        # depthwise conv as diagonal matmul weights; rev uses flipped taps
        dwd = np.zeros((2, 2, 4, 128, 128), np.float32)
        for di, s in enumerate(("f", "r")):
            w = f32(inputs[f"cv_w_{s}"])[rows, 0]     # (256, 4)
            for dt in range(2):
                for k in range(4):
                    tap = w[dt * 128:(dt + 1) * 128, k if di == 0 else 3 - k]
                    dwd[di, dt, k][np.arange(128), np.arange(128)] = tap
        cvb_cols = [f32(inputs[f"cv_b_{s}"])[rows][dt * 128:(dt + 1) * 128]
                    .reshape(128, 1) for s in ("f", "r") for dt in range(2)]
        D_cols = [f32(inputs[f"D_{s}"])[rows][dt * 128:(dt + 1) * 128]
                  .reshape(128, 1) for s in ("f", "r") for dt in range(2)]
        colpack = np.ascontiguousarray(np.concatenate(
            common_cols + dtb_cols + cvb_cols + D_cols, axis=1).astype(np.float32))
        outTm = bf(opw[:, rows].T.reshape(2, 128, DM))
        xpadded = bf(np.pad(x[b], ((0, 0), (1, 1))))
        m = dict(common)
        m.update(x=xpadded, inprojT=inprojT, augT=augTm, xpT=xpTm, dtT=dtTm,
                 colpack=colpack, dwdiag=bf(dwd), outT=outTm)
        in_maps.append(m)
    return a_vals, in_maps


def kernel(**inputs) -> np.ndarray:
    from concourse.bass_utils import run_bass_kernel_spmd
    a_vals, in_maps = prep_inputs(inputs)
    nc = build_program(a_vals)
    res = run_bass_kernel_spmd(nc, in_maps, list(range(NCORES)))
    out = np.stack([res.results[0]["out"], res.results[NGRP]["out"]])
    return np.ascontiguousarray(out.astype(np.float32))


if __name__ == "__main__":
    import reference as R
    import jax
    with jax.default_device(jax.devices("cpu")[0]):
        inp = {k: np.asarray(v) for k, v in R.setup_inputs().items()}
        ref = np.asarray(R.reference(**R.setup_inputs()))
    got = kernel(**inp)
    err = np.abs(got - ref).max() / np.abs(ref).max()
    print("Relative error:", err)
